# revision 5
# baseline (speedup 1.0000x reference)
"""Trainium2 Bass kernel for nn_ConstraintLoss (grid second-difference loss).

Contract: kernel(theta, grid_size) takes the FULL inputs (theta [512,16384,2]
fp32, grid_size == 128) and returns the FULL output (scalar fp32 loss),
sharding batch-parallel across 8 NeuronCores internally.

Math (n=128, B=512, g = theta.reshape(B,n,n,2)):
  row terms: second difference along i of (first diff along i)^2, abs,
  clamped at D_FLOOR=0.08, averaged over (B, n, n-2, 2ch).
  col terms: same along j.
  grad terms (batch element 0 only): sums of |cross products| along i / j,
  floored at G_FLOOR=0.02.

Device layout per core (64-batch shard):
  partition p = h*64 + b  (h in {0,1}, b in [0,64)) holds grid rows
  [h*62, h*62+66) of g[b], flattened (row, i, ch) -> 66*256 elements.
  fp32 is DMA'd in 4 slot-chunks, converted to fp16 (GpSimd) into one
  persistent SBUF tile. Row/col paths: shifted-AP subtract (DVE, fp16 2x),
  square (ACT), subtract (DVE), then clamp+sum via TWO fused
  tensor_scalar+accumulate passes using
      sum max(F,|x|) = sum max(x,F) - sum min(x,-F) - count*F
  (the ISA's TensorScalarCacheReduce does not support abs_max).
  Row path runs on flattened chunks including cross-row "junk" positions;
  junk is re-summed by tiny strided bypass+accum ops and subtracted on the
  host. Rows 62..65 appear in both halves; a small window op recomputes their
  contribution (h=0 partitions) for host-side subtraction. Col path has no
  junk; col outputs j=62,63 are double-counted and similarly corrected.
  Grad terms are computed in fp32 from small extra loads of g[0] (valid on
  core 0 only).

Host combine: fp64 reduction of the per-partition stats columns plus the
statically-known count*F terms.
"""

import numpy as np

import concourse.bacc as bacc
import concourse.bass as bass
import concourse.tile as tile
from concourse import mybir
from concourse.bass_utils import run_bass_kernel_spmd

F16 = mybir.dt.float16
F32 = mybir.dt.float32
ALU = mybir.AluOpType
ACTF = mybir.ActivationFunctionType

N = 128                # grid size
RB = 2 * N             # elements per grid row (i,ch interleaved) = 256
SLOTS = 66             # grid rows held per partition (64 + 2 halo)
HSTEP = 62 * RB        # DRAM element step between h=0 and h=1 row windows
BSTRIDE = N * N * 2    # DRAM element step between batch elements = 32768
BPC = 64               # batch elements per core
D_FLOOR = 0.08
G_FLOOR = 0.02

# slot-chunk boundaries for DMA/conv/row path
CHUNKS = [(0, 16), (16, 32), (32, 48), (48, 66)]
# col path d2c output ranges (slot-diff index s); dc needs slots [s0, s1+1]
COL_CHUNKS = [(0, 16), (16, 32), (32, 48), (48, 64)]

# stats columns: every accumulation site has an A (max) and B (min) column
NSTAT = 34
C_ROWA, C_ROWB = 0, 4          # 4 row-main chunks
C_ROWJA, C_ROWJB = 8, 12       # 4 row-junk chunks
C_RCA, C_RCB = 16, 17          # row dup-window main (h=0 partitions)
C_RCJA, C_RCJB = 18, 19        # row dup-window junk
C_COLA, C_COLB = 20, 24        # 4 col-main chunks
C_CCA, C_CCB = 28, 29          # col dup-window
C_GRA, C_GRB = 30, 31          # grad row
C_GCA, C_GCB = 32, 33          # grad col


def build_tile_kernel(tc, stats, theta):
    """Emit the Tile program. stats: [128, NSTAT] f32 out, theta: [64,16384,2] f32 in."""
    nc = tc.nc
    th = theta.tensor

    from contextlib import ExitStack

    def clamp_accum(pool_out_ap, x_ap, floor, col_a, col_b, junk=None):
        """Accumulate sum(max(floor,|x|)) decomposed as max/min passes.
        pool_out_ap: scratch AP (same shape as x) that pass A may clobber.
        x is clobbered by pass B. junk: (strided_view_fn, col_ja, col_jb)."""
        nc.vector.tensor_scalar(
            out=pool_out_ap, in0=x_ap, scalar1=float(floor), scalar2=None,
            op0=ALU.max, op1=ALU.add, accum_out=col_a,
        )
        if junk is not None:
            va, col_ja, col_jb = junk
            ja = va(pool_out_ap)
            nc.vector.tensor_scalar(
                out=ja, in0=ja, scalar1=0.0, scalar2=None,
                op0=ALU.bypass, op1=ALU.add, accum_out=col_ja,
            )
        nc.vector.tensor_scalar(
            out=x_ap, in0=x_ap, scalar1=float(-floor), scalar2=None,
            op0=ALU.min, op1=ALU.add, accum_out=col_b,
        )
        if junk is not None:
            va, col_ja, col_jb = junk
            jb = va(x_ap)
            nc.vector.tensor_scalar(
                out=jb, in0=jb, scalar1=0.0, scalar2=None,
                op0=ALU.bypass, op1=ALU.add, accum_out=col_jb,
            )

    with ExitStack() as ctx:
        pool_f32 = ctx.enter_context(tc.tile_pool(name="f32in", bufs=2))
        pool_t16 = ctx.enter_context(tc.tile_pool(name="t16", bufs=1))
        pool_d1 = ctx.enter_context(tc.tile_pool(name="d1", bufs=2))
        pool_d2 = ctx.enter_context(tc.tile_pool(name="d2", bufs=2))
        pool_dc = ctx.enter_context(tc.tile_pool(name="dc", bufs=2))
        pool_d2c = ctx.enter_context(tc.tile_pool(name="d2c", bufs=2))
        pool_small = ctx.enter_context(tc.tile_pool(name="small", bufs=1))
        pool_stat = ctx.enter_context(tc.tile_pool(name="stat", bufs=1))
        stats_sb = pool_stat.tile([128, NSTAT], F32)
        nc.vector.memset(stats_sb, 0.0)

        # persistent fp16 copy of the shard, (h,b)-partitioned
        t16 = pool_t16.tile([128, SLOTS * RB], F16)

        def scol(c):
            return stats_sb[:, c:c + 1]

        # ---- load + convert chunks
        for (s0, s1) in CHUNKS:
            L = (s1 - s0) * RB
            f32c = pool_f32.tile([128, 18 * RB], F32, tag="f32c")
            src = bass.AP(
                tensor=th,
                offset=s0 * RB,
                ap=[[HSTEP, 2], [BSTRIDE, BPC], [1, L]],
            )
            nc.sync.dma_start(out=f32c[:, :L], in_=src)
            nc.gpsimd.tensor_copy(t16[:, s0 * RB:s1 * RB], f32c[:, :L])

        # ---- row path (per chunk, flattened; within-row shifts)
        for ci, (s0, s1) in enumerate(CHUNKS):
            Nc = (s1 - s0) * RB
            base = s0 * RB
            R = s1 - s0
            d1 = pool_d1.tile([128, 18 * RB], F16, tag="d1")
            nc.vector.tensor_sub(
                d1[:, :Nc - 2], t16[:, base + 2:base + Nc], t16[:, base:base + Nc - 2]
            )
            nc.scalar.activation(d1[:, :Nc - 2], d1[:, :Nc - 2], ACTF.Square)
            d2 = pool_d2.tile([128, 18 * RB], F16, tag="d2")
            nc.vector.tensor_sub(
                d2[:, :Nc - 4], d1[:, 2:Nc - 2], d1[:, :Nc - 4]
            )

            def junkview(ap, R=R):
                return ap[:, 252:252 + (R - 1) * RB].rearrange(
                    "p (r e) -> p r e", e=RB
                )[:, :, 0:4]

            clamp_accum(
                d1[:, :Nc - 4], d2[:, :Nc - 4], D_FLOOR,
                scol(C_ROWA + ci), scol(C_ROWB + ci),
                junk=(junkview, scol(C_ROWJA + ci), scol(C_ROWJB + ci)),
            )

        # ---- col path (slot-direction shifts; no junk)
        for ci, (s0, s1) in enumerate(COL_CHUNKS):
            nd = s1 - s0            # d2c outputs (16)
            Md = (nd + 1) * RB      # dc elements (17*256)
            dc = pool_dc.tile([128, 17 * RB], F16, tag="dc")
            nc.vector.tensor_sub(
                dc[:, :Md],
                t16[:, (s0 + 1) * RB:(s0 + nd + 2) * RB],
                t16[:, s0 * RB:(s0 + nd + 1) * RB],
            )
            nc.scalar.activation(dc[:, :Md], dc[:, :Md], ACTF.Square)
            d2c = pool_d2c.tile([128, 16 * RB], F16, tag="d2c")
            nc.vector.tensor_sub(
                d2c[:, :nd * RB], dc[:, RB:Md], dc[:, :nd * RB]
            )
            clamp_accum(
                dc[:, :nd * RB], d2c[:, :nd * RB], D_FLOOR,
                scol(C_COLA + ci), scol(C_COLB + ci),
            )

        # ---- duplicate-window corrections (rows 62..65 live on h=0 slots 62..65)
        W = t16[0:64, 62 * RB:66 * RB]   # [64, 1024]
        # row window
        wd1 = pool_small.tile([64, 1022], F16, tag="wd1")
        nc.vector.tensor_sub(wd1, W[:, 2:1024], W[:, 0:1022])
        nc.scalar.activation(wd1, wd1, ACTF.Square)
        wd2 = pool_small.tile([64, 1020], F16, tag="wd2")
        nc.vector.tensor_sub(wd2, wd1[:, 2:1022], wd1[:, 0:1020])

        def wjunkview(ap):
            return ap[:, 252:252 + 3 * RB].rearrange("p (r e) -> p r e", e=RB)[:, :, 0:4]

        clamp_accum(
            wd1[:, :1020], wd2[:, :1020], D_FLOOR,
            stats_sb[0:64, C_RCA:C_RCA + 1], stats_sb[0:64, C_RCB:C_RCB + 1],
            junk=(wjunkview, stats_sb[0:64, C_RCJA:C_RCJA + 1],
                  stats_sb[0:64, C_RCJB:C_RCJB + 1]),
        )
        # col window (col outputs j=62,63)
        cwdc = pool_small.tile([64, 768], F16, tag="cwdc")
        nc.vector.tensor_sub(cwdc, W[:, RB:1024], W[:, 0:768])
        nc.scalar.activation(cwdc, cwdc, ACTF.Square)
        cwd2 = pool_small.tile([64, 512], F16, tag="cwd2")
        nc.vector.tensor_sub(cwd2, cwdc[:, RB:768], cwdc[:, 0:512])
        clamp_accum(
            cwdc[:, :512], cwd2[:, :512], D_FLOOR,
            stats_sb[0:64, C_CCA:C_CCA + 1], stats_sb[0:64, C_CCB:C_CCB + 1],
        )

        # ---- grad terms (fp32, from g[0]; meaningful on core 0 only)
        with tc.tile_pool(name="grad", bufs=1) as gp:
            T = gp.tile([128, RB], F32)
            nc.sync.dma_start(out=T, in_=bass.AP(tensor=th, offset=0, ap=[[RB, 128], [1, RB]]))
            T1 = gp.tile([126, RB], F32)
            nc.sync.dma_start(out=T1, in_=bass.AP(tensor=th, offset=RB, ap=[[RB, 126], [1, RB]]))
            T2 = gp.tile([126, RB], F32)
            nc.sync.dma_start(out=T2, in_=bass.AP(tensor=th, offset=2 * RB, ap=[[RB, 126], [1, RB]]))

            Tc = T.rearrange("p (i c) -> p c i", c=2)

            def x_(a, b):
                return Tc[:, 0:1, a:b].squeeze(1)

            def y_(a, b):
                return Tc[:, 1:2, a:b].squeeze(1)

            # row grad: vary i within partitions
            A = gp.tile([128, 126], F32)
            B_ = gp.tile([128, 126], F32)
            C_ = gp.tile([128, 126], F32)
            D_ = gp.tile([128, 126], F32)
            nc.any.tensor_sub(A, y_(1, 127), y_(0, 126))
            nc.any.tensor_sub(B_, x_(1, 127), x_(2, 128))
            nc.any.tensor_sub(C_, y_(1, 127), y_(2, 128))
            nc.any.tensor_sub(D_, x_(1, 127), x_(0, 126))
            nc.any.tensor_mul(A, A, B_)
            nc.any.tensor_mul(C_, C_, D_)
            nc.any.tensor_sub(A, A, C_)
            clamp_accum(B_, A, 0.0, scol(C_GRA), scol(C_GRB))

            # col grad: vary j across partition-shifted copies
            T0c = T[0:126, :].rearrange("p (i c) -> p c i", c=2)
            T1c = T1.rearrange("p (i c) -> p c i", c=2)
            T2c = T2.rearrange("p (i c) -> p c i", c=2)

            def uch(t, c):
                return t[:, c:c + 1, :].squeeze(1)

            A2 = gp.tile([126, 128], F32)
            B2 = gp.tile([126, 128], F32)
            C2 = gp.tile([126, 128], F32)
            D2 = gp.tile([126, 128], F32)
            nc.any.tensor_sub(A2, uch(T1c, 1), uch(T0c, 1))
            nc.any.tensor_sub(B2, uch(T1c, 0), uch(T2c, 0))
            nc.any.tensor_sub(C2, uch(T1c, 1), uch(T2c, 1))
            nc.any.tensor_sub(D2, uch(T1c, 0), uch(T0c, 0))
            nc.any.tensor_mul(A2, A2, B2)
            nc.any.tensor_mul(C2, C2, D2)
            nc.any.tensor_sub(A2, A2, C2)
            clamp_accum(
                B2, A2, 0.0,
                stats_sb[0:126, C_GCA:C_GCA + 1], stats_sb[0:126, C_GCB:C_GCB + 1],
            )

        # ---- write out
        nc.sync.dma_start(out=stats, in_=stats_sb)


_PROGRAM = None


def _get_program():
    global _PROGRAM
    if _PROGRAM is None:
        nc = bacc.Bacc("TRN2", target_bir_lowering=False, debug=False)
        theta = nc.dram_tensor("theta", [BPC, N * N, 2], F32, kind="ExternalInput").ap()
        stats = nc.dram_tensor("stats", [128, NSTAT], F32, kind="ExternalOutput").ap()
        with tile.TileContext(nc) as tc:
            build_tile_kernel(tc, stats, theta)
        nc.compile()
        _PROGRAM = nc
    return _PROGRAM


# per-core element counts for the count*F terms of the max/min decomposition
_CNT_ROW = 128 * (3 * 4092 + 4604) - 128 * (3 * 60 + 68) - 64 * (1020 - 12)
_CNT_COL = 128 * 4 * 4096 - 64 * 512


def combine_stats(stats_list):
    """Host-side reduction of per-core stats -> scalar loss (fp64)."""
    s = [np.asarray(x, np.float64) for x in stats_list]

    def ab(si, ca, cb, k=1):
        return (si[:, ca:ca + k] - si[:, cb:cb + k]).sum()

    row = sum(
        ab(si, C_ROWA, C_ROWB, 4) - ab(si, C_ROWJA, C_ROWJB, 4)
        - (ab(si, C_RCA, C_RCB) - ab(si, C_RCJA, C_RCJB))
        for si in s
    ) - len(s) * D_FLOOR * _CNT_ROW
    col = sum(
        ab(si, C_COLA, C_COLB, 4) - ab(si, C_CCA, C_CCB) for si in s
    ) - len(s) * D_FLOOR * _CNT_COL
    rg = ab(s[0], C_GRA, C_GRB)
    cg = ab(s[0], C_GCA, C_GCB)
    denom = 512 * N * (N - 2)
    return (row + col) / denom + max(rg, G_FLOOR) + max(cg, G_FLOOR)


def _run(theta, trace=False):
    theta = np.ascontiguousarray(np.asarray(theta, dtype=np.float32))
    assert theta.shape == (512, N * N, 2), theta.shape
    nc = _get_program()
    in_maps = [{"theta": theta[k * BPC:(k + 1) * BPC]} for k in range(8)]
    res = run_bass_kernel_spmd(nc, in_maps, list(range(8)), trace=trace)
    loss = combine_stats([r["stats"] for r in res.results])
    return loss, res


def kernel(theta, grid_size):
    assert int(grid_size) == N, grid_size
    loss, _ = _run(theta)
    return np.float32(loss)
